# revision 24
# baseline (speedup 1.0000x reference)
"""Bahdanau attention w/ coverage — TRN2 distributed kernel (8 NeuronCores).

Sharding: pure data parallel. B=32 batches -> 4 per core. Weights replicated.
Host passes enc transposed per-core (layout choice): encT [4, N=1024, S=4096].

Per core, per batch b:
  F[s,m]   = sum_n encT[n,s]*WhT[n,m]  (bf16 PE matmul, K=n on partitions)
           + cov[s]*wc[m] + (bh+bs+dec@Ws.T)[m]   (K=2 rank-1 matmul into same PSUM)
  T        = tanh(F)                    (ACT, PSUM->SBUF bf16)
  e[s]     = sum_m T[s,m]*v[m]          (DVE tensor_tensor_reduce)
  attn     = exp(e)*mask / sum(exp(e)*mask)   (no max-subtract: |e|<=sum|v|~26)
  ctx[n]   = sum_s attn[s]*encT[n,s]    (DVE reduce vs resident encT tiles)
  cov_out  = cov + attn
"""

import numpy as np

import concourse.bass as bass
import concourse.mybir as mybir
import concourse.tile as tile
from concourse import bacc
from concourse.bass_utils import run_bass_kernel_spmd
from concourse.masks import make_identity

B, S, N = 32, 4096, 1024
NCORES = 8
BPC = B // NCORES          # batches per core
NCH = N // 128             # 8 n-chunks
NSB = S // 128             # 32 s-blocks per batch
MH = 2                     # m halves (512 each)

F32 = mybir.dt.float32
BF16 = mybir.dt.bfloat16
I32 = mybir.dt.int32
AX = mybir.AxisListType
OP = mybir.AluOpType
AF = mybir.ActivationFunctionType

_CACHED = {}


def build_nc(BPC=BPC, S=S, N=N):
    NCH = N // 128
    NSB = S // 128
    nc = bacc.Bacc(None, target_bir_lowering=False)

    encT = nc.declare_dram_parameter("encT", [BPC, N, S], F32, False)
    decT = nc.declare_dram_parameter("decT", [N, BPC], F32, False)
    cov = nc.declare_dram_parameter("cov", [BPC, S], F32, False)
    lens = nc.declare_dram_parameter("lens", [1, BPC], I32, False)
    WhT = nc.declare_dram_parameter("WhT", [N, N], F32, False)
    WsT = nc.declare_dram_parameter("WsT", [N, N], F32, False)
    bhbs = nc.declare_dram_parameter("bhbs", [1, N], F32, False)
    wc = nc.declare_dram_parameter("wc", [1, N], F32, False)
    v = nc.declare_dram_parameter("v", [1, N], F32, False)
    onesrow = nc.declare_dram_parameter("onesrow", [1, S], F32, False)

    out_ctx = nc.declare_dram_parameter("out_ctx", [BPC, N], F32, True)
    out_attn = nc.declare_dram_parameter("out_attn", [BPC, S], F32, True)
    out_cov = nc.declare_dram_parameter("out_cov", [BPC, S], F32, True)
    attn_bounce = nc.dram_tensor("attn_bounce", [BPC, S], F32)

    with tile.TileContext(nc) as tc:
        with (
            tc.tile_pool(name="const", bufs=1) as const,
            tc.tile_pool(name="chunks", bufs=14) as chunks,
            tc.tile_pool(name="tt", bufs=3) as ttp,
            tc.tile_pool(name="scr", bufs=2) as scrp,
            tc.tile_pool(name="big", bufs=1) as bigp,
            tc.tile_pool(name="perb", bufs=2) as perb,
            tc.tile_pool(name="small", bufs=4) as small,
            tc.tile_pool(name="psF", bufs=2, space="PSUM") as psF,
            tc.tile_pool(name="psS", bufs=3, space="PSUM") as psS,
        ):
            # ---- static constants ----
            whts = const.tile([128, NCH, N], BF16, tag="wht")
            for k in range(NCH):
                nc.gpsimd.dma_start(out=whts[:, k, :], in_=WhT[128 * k:128 * k + 128, :])

            ident = const.tile([128, 128], F32, tag="ident")
            make_identity(nc, ident)

            v_f32 = const.tile([128, N], F32, tag="vf32")
            va = v[:, :]
            nc.gpsimd.dma_start(
                out=v_f32,
                in_=bass.AP(tensor=va.tensor, offset=va.offset, ap=[[0, 128], [1, N]]),
            )
            v_full = const.tile([128, N], BF16, tag="vfull")
            nc.vector.tensor_copy(v_full, v_f32)

            ones_sq = const.tile([128, 128], F32, tag="ones")
            nc.vector.memset(ones_sq, 1.0)

            iota_f = const.tile([128, NSB], F32, tag="iotaf")
            iota_i = const.tile([128, NSB], I32, tag="iotai")
            nc.gpsimd.iota(iota_i, pattern=[[128, NSB]], base=0, channel_multiplier=1)
            nc.vector.tensor_copy(iota_f, iota_i)

            lens_i = const.tile([128, BPC], I32, tag="lensi")
            la = lens[:, :]
            nc.gpsimd.dma_start(
                out=lens_i,
                in_=bass.AP(tensor=la.tensor, offset=la.offset, ap=[[0, 128], [1, BPC]]))
            lens_bc = const.tile([128, BPC], F32, tag="lensbc")
            nc.vector.tensor_copy(lens_bc, lens_i)

            # ---- dec_feats = dec @ Ws.T (+ bh + bs) for all 4 batches ----
            decT_sb = const.tile([128, NCH, BPC], BF16, tag="decT")
            for k in range(NCH):
                nc.gpsimd.dma_start(out=decT_sb[:, k, :], in_=decT[128 * k:128 * k + 128, :])
            # WsT chunks go through the big chunk pool transiently
            ps_dec = psF.tile([BPC, N], F32, tag="F")
            for k in range(NCH):
                wst_k = chunks.tile([128, N], BF16, tag="enc")
                nc.gpsimd.dma_start(out=wst_k, in_=WsT[128 * k:128 * k + 128, :])
                for mh in range(MH):
                    nc.tensor.matmul(
                        ps_dec[:, mh * 512:(mh + 1) * 512],
                        lhsT=decT_sb[:, k, :],
                        rhs=wst_k[:, mh * 512:(mh + 1) * 512],
                        start=(k == 0), stop=(k == NCH - 1),
                    )
            bias_rows = const.tile([BPC, N], F32, tag="biasrows")
            bhbs_b = const.tile([BPC, N], F32, tag="bhbsb")
            ba = bhbs[:, :]
            nc.gpsimd.dma_start(
                out=bhbs_b,
                in_=bass.AP(tensor=ba.tensor, offset=ba.offset, ap=[[0, BPC], [1, N]]))
            nc.vector.tensor_add(bias_rows, ps_dec, bhbs_b)

            # ---- per batch ----
            for b in range(BPC):
                # rank-1 operands: lhsT rows {cov(bf16), ones}; rhs rows {wc, bias_b}
                onescov = perb.tile([2, S], BF16, tag="onescov")
                nc.gpsimd.dma_start(out=onescov[0:1, :], in_=cov[b:b + 1, :])
                nc.gpsimd.dma_start(out=onescov[1:2, :], in_=onesrow[:, :])
                r1rhs = perb.tile([2, N], BF16, tag="r1rhs")
                nc.gpsimd.dma_start(out=r1rhs[0:1, :], in_=wc[:, :])
                nc.gpsimd.dma_start(out=r1rhs[1:2, :], in_=bias_rows[b:b + 1, :])

                cov_sm = perb.tile([NSB, 128], F32, tag="covsm")
                nc.gpsimd.dma_start(
                    out=cov_sm, in_=cov[b, :].rearrange("(p f) -> p f", p=NSB))

                enc_b = []
                for k in range(NCH):
                    ek = chunks.tile([128, S], BF16, tag="enc")
                    nc.gpsimd.dma_start(out=ek, in_=encT[b, 128 * k:128 * k + 128, :])
                    enc_b.append(ek)

                e_buf = perb.tile([128, NSB], F32, tag="ebuf")

                for sb in range(NSB):
                    F = psF.tile([128, N], F32, tag="F")
                    sl = slice(128 * sb, 128 * (sb + 1))
                    for mh in range(MH):
                        nc.tensor.matmul(
                            F[:, mh * 512:(mh + 1) * 512],
                            lhsT=onescov[:, sl],
                            rhs=r1rhs[:, mh * 512:(mh + 1) * 512],
                            start=True, stop=False,
                        )
                    for k in range(NCH):
                        for mh in range(MH):
                            nc.tensor.matmul(
                                F[:, mh * 512:(mh + 1) * 512],
                                lhsT=enc_b[k][:, sl],
                                rhs=whts[:, k, mh * 512:(mh + 1) * 512],
                                start=False, stop=(k == NCH - 1),
                            )
                    T = ttp.tile([128, N], BF16, tag="T")
                    nc.scalar.activation(T, F, AF.Tanh)
                    scr = scrp.tile([128, N], BF16, tag="scr")
                    nc.vector.scalar_tensor_tensor(
                        out=scr, in0=T, scalar=1.0, in1=v_full,
                        op0=OP.mult, op1=OP.mult,
                        accum_out=e_buf[:, sb:sb + 1],
                    )

                # ---- softmax (no max-subtraction; tanh bounds |e|) ----
                expw = small.tile([128, NSB], F32, tag="expw")
                nc.scalar.activation(expw, e_buf, AF.Exp)
                mask = small.tile([128, NSB], F32, tag="mask")
                nc.vector.tensor_scalar(
                    out=mask, in0=iota_f, scalar1=lens_bc[:, b:b + 1], scalar2=None,
                    op0=OP.is_lt)
                w = small.tile([128, NSB], F32, tag="w")
                nc.vector.tensor_mul(w, expw, mask)
                ps_z = psS.tile([128, NSB], F32, tag="ps_small")
                nc.tensor.matmul(ps_z, lhsT=ones_sq, rhs=w, start=True, stop=True)
                zsum = small.tile([128, 1], F32, tag="zsum")
                nc.vector.reduce_sum(zsum, ps_z, axis=AX.X)
                zinv_bc = small.tile([128, 1], F32, tag="zinvbc")
                nc.vector.reciprocal(zinv_bc, zsum)
                attn_pm = small.tile([128, NSB], F32, tag="attnpm")
                nc.vector.tensor_scalar_mul(attn_pm, w, zinv_bc[:, 0:1])

                # ---- attn/coverage outputs (s-major [32,128] layout) ----
                ps_t = psS.tile([NSB, 128], F32, tag="ps_small")
                nc.tensor.transpose(ps_t, attn_pm, ident)
                attn_sm = perb.tile([NSB, 128], F32, tag="attnsm")
                nc.vector.tensor_copy(attn_sm, ps_t)
                cov_o = perb.tile([NSB, 128], F32, tag="covo")
                nc.vector.tensor_add(cov_o, attn_sm, cov_sm)
                nc.gpsimd.dma_start(
                    out=out_attn[b, :].rearrange("(p f) -> p f", p=NSB), in_=attn_sm)
                nc.gpsimd.dma_start(
                    out=out_cov[b, :].rearrange("(p f) -> p f", p=NSB), in_=cov_o)
                nc.gpsimd.dma_start(
                    out=attn_bounce[b, :].rearrange("(p f) -> p f", p=NSB), in_=attn_sm)

                # ---- context: broadcast attn from DRAM, fused mul-reduce ----
                attn_full = bigp.tile([128, S], BF16, tag="attnfull")
                oa = attn_bounce[b, :]
                nc.gpsimd.dma_start(
                    out=attn_full,
                    in_=bass.AP(tensor=oa.tensor, offset=oa.offset, ap=[[0, 128], [1, S]]),
                )
                ctx_cols = perb.tile([128, NCH], F32, tag="ctxcols")
                scr2 = bigp.tile([128, S], BF16, tag="scr2")
                for k in range(NCH):
                    nc.vector.scalar_tensor_tensor(
                        out=scr2, in0=enc_b[k], scalar=1.0, in1=attn_full,
                        op0=OP.mult, op1=OP.mult,
                        accum_out=ctx_cols[:, k:k + 1],
                    )
                ps_c = psS.tile([NCH, 128], F32, tag="ps_small")
                nc.tensor.transpose(ps_c, ctx_cols, ident)
                ctx_sm = small.tile([NCH, 128], F32, tag="ctxsm")
                nc.vector.tensor_copy(ctx_sm, ps_c)
                nc.gpsimd.dma_start(
                    out=out_ctx[b, :].rearrange("(p f) -> p f", p=NCH), in_=ctx_sm)
    nc.compile()
    return nc


def _prep_in_maps(dec_in_state, enc_states, enc_lens, coverage_vector, Wh, bh, Ws, bs, wc, v):
    dec_in_state = np.asarray(dec_in_state, np.float32)
    enc_states = np.asarray(enc_states, np.float32)
    enc_lens = np.asarray(enc_lens).astype(np.int32)
    coverage_vector = np.asarray(coverage_vector, np.float32)
    WhT = np.ascontiguousarray(np.asarray(Wh, np.float32).T)
    WsT = np.ascontiguousarray(np.asarray(Ws, np.float32).T)
    bhbs = (np.asarray(bh, np.float32) + np.asarray(bs, np.float32)).reshape(1, N)
    wc_r = np.asarray(wc, np.float32).reshape(1, N)
    v_r = np.asarray(v, np.float32).reshape(1, N)

    in_maps = []
    for c in range(NCORES):
        sl = slice(c * BPC, (c + 1) * BPC)
        in_maps.append({
            "encT": np.ascontiguousarray(enc_states[sl].transpose(0, 2, 1)),
            "decT": np.ascontiguousarray(dec_in_state[sl].T),
            "cov": np.ascontiguousarray(coverage_vector[sl]),
            "lens": np.ascontiguousarray(enc_lens[sl].reshape(1, BPC)),
            "WhT": WhT, "WsT": WsT, "bhbs": bhbs, "wc": wc_r, "v": v_r,
            "onesrow": np.ones((1, S), np.float32),
        })
    return in_maps


def kernel(dec_in_state, enc_states, enc_lens, coverage_vector, Wh, bh, Ws, bs, wc, v):
    if "nc" not in _CACHED:
        _CACHED["nc"] = build_nc()
    nc = _CACHED["nc"]
    in_maps = _prep_in_maps(dec_in_state, enc_states, enc_lens, coverage_vector,
                            Wh, bh, Ws, bs, wc, v)
    res = run_bass_kernel_spmd(nc, in_maps, core_ids=list(range(NCORES))).results
    context = np.concatenate([r["out_ctx"] for r in res], axis=0)
    attn = np.concatenate([r["out_attn"] for r in res], axis=0)
    cov_out = np.concatenate([r["out_cov"] for r in res], axis=0)
    return (context, attn, cov_out)
